# revision 2
# baseline (speedup 1.0000x reference)
"""Trainium2 Bass kernel for nn_DynaResidualBlock (hypernetwork residual block).

Reference computation (B=32, LAT=256, FIN=FOUT=32, FH=64, H=W=128):
    h  = lat @ W1 + b1                       # [B, 9408]
    ks = h @ W2 + b2                         # [B, 9408]  (W2 is 9408x9408 = 354 MB)
    per-sample 1x1 convs with kernels/biases sliced out of ks:
    x_s = k_short(x) ; y = k_out(lrelu(k_mid(lrelu(k_in(x))))) + x_s

Sharding over 8 cores:
  - hypernet contraction dim (9408) split 1176-per-core: core i holds
    W1[:, shard_i] and W2[shard_i, :] and computes a partial ks for ALL
    32 samples; a ReduceScatter then hands core i the summed ks rows for
    its own 4 samples.
  - conv phase is data-parallel: core i processes samples 4i..4i+3, packed
    as 2 sample-pairs with block-diagonal weight matrices so each 1x1 conv
    over a 512-pixel tile is a single PE matmul.

Host-side layout tricks (pure data reformatting, no reference FLOPs):
  - W2's columns are permuted so each generated conv kernel lands in
    SBUF already transposed into the PE's lhsT layout (no on-chip
    transposes needed).
  - x gets 3 constant "ones" channels per pair so conv biases ride in as
    extra matmul rows (b_in, b_short, b_out fold into the matmuls).
  - lat is passed pre-transposed; b2 rides as an extra W2 row on core 0.
"""

import contextlib

import numpy as np

import concourse.bacc as bacc
import concourse.mybir as mybir
import concourse.tile as tile
from concourse.bass_utils import run_bass_kernel_spmd

N_CORES = 8
B, LAT, FIN, FOUT, FH, H, W = 32, 256, 32, 32, 64, 128, 128
HW = H * W
K_IN, K_MID, K_OUT, K_SH = FH * FIN, FH * FH, FOUT * FH, FOUT * FIN
K_TOT = K_IN + K_MID + K_OUT + K_SH + FH + FH + FOUT + FOUT  # 9408
SHARD = K_TOT // N_CORES  # 1176 hypernet columns per core
KP = SHARD + 1            # + one bias row (b2, on core 0 only)
KPAD = 1280               # h length padded to 10 chunks of 128
NCH = KPAD // 128         # 10
BPC = B // N_CORES        # 4 samples per core
XC = 2 * FIN + 3          # 67 = 2x32 x-channels + 3 ones-channels per pair
F32 = mybir.dt.float32

OFF_IN, OFF_MID = 0, K_IN
OFF_OUT, OFF_SHC = K_IN + K_MID, K_IN + K_MID + K_OUT
OFF_B = OFF_SHC + K_SH  # 9216: b_in 64 | b_mid 64 | b_out 32 | b_short 32
OB_IN, OB_MID = OFF_B, OFF_B + FH
OB_OUT, OB_SH = OFF_B + 2 * FH, OFF_B + 2 * FH + FOUT

# ks column groups for phase A (align with conv-weight segment boundaries)
GROUPS = [(0, 2048), (2048, 2048), (4096, 2048), (6144, 2048), (8192, 1216)]

# columns of y1's leaky-relu computed on DVE (rest on ACT); 0 = all on ACT.
# NOTE: the DVE path needs two PSUM reads in one op, which walrus rejects —
# keep 0 unless restructured.
DVE_COLS = 0

_CACHE: dict = {}


def _build():
    nc = bacc.Bacc("TRN2", target_bir_lowering=False, num_devices=N_CORES)
    AF = mybir.ActivationFunctionType

    xs = nc.dram_tensor("xs", [2, XC, HW], F32, kind="ExternalInput")
    latT = nc.dram_tensor("latT", [LAT, B], F32, kind="ExternalInput")
    w1s = nc.dram_tensor("w1s", [LAT, KPAD], F32, kind="ExternalInput")
    b1s = nc.dram_tensor("b1s", [128, NCH], F32, kind="ExternalInput")
    w2s = nc.dram_tensor("w2s", [KP, K_TOT], F32, kind="ExternalInput")
    out = nc.dram_tensor("out", [2, 2 * FOUT, HW], F32, kind="ExternalOutput")

    ks_part = nc.dram_tensor("ks_part", [B, K_TOT], F32)
    ks_own = nc.dram_tensor("ks_own", [BPC, K_TOT], F32)

    with tile.TileContext(nc) as tc, contextlib.ExitStack() as ctx:
        sing = ctx.enter_context(tc.tile_pool(name="sing", bufs=1))

        # ---------------- Phase A: hypernet (partial ks for all B) --------
        with tc.tile_pool(name="psA", bufs=2, space="PSUM") as psA, \
             tc.tile_pool(name="psK", bufs=6, space="PSUM") as psK, \
             tc.tile_pool(name="w2pool", bufs=4) as w2pool, \
             tc.tile_pool(name="kspool", bufs=2) as kspool:
            latT_sb = sing.tile([128, 2 * B], F32)
            w1_sb = sing.tile([128, 2 * KPAD], F32)
            b1_sb = sing.tile([128, NCH], F32)
            hT_sb = sing.tile([128, NCH * B], F32)
            for l in range(2):
                nc.sync.dma_start(out=latT_sb[:, l * B:(l + 1) * B],
                                  in_=latT[128 * l:128 * (l + 1), :])
                nc.sync.dma_start(out=w1_sb[:, l * KPAD:(l + 1) * KPAD],
                                  in_=w1s[128 * l:128 * (l + 1), :])
            nc.sync.dma_start(out=b1_sb[:, :], in_=b1s[:, :])

            # hT[kcol, b] = sum_l W1[l, kcol] * lat[b, l]  (+ b1[kcol])
            for c in range(NCH):
                ph = psA.tile([128, B], F32, tag="ph", name=f"ph{c}")
                for l in range(2):
                    nc.tensor.matmul(
                        ph,
                        lhsT=w1_sb[:, l * KPAD + 128 * c: l * KPAD + 128 * c + 128],
                        rhs=latT_sb[:, l * B:(l + 1) * B],
                        start=(l == 0), stop=(l == 1))
                nc.scalar.activation(out=hT_sb[:, c * B:(c + 1) * B], in_=ph,
                                     func=AF.Identity, bias=b1_sb[:, c:c + 1],
                                     scale=1.0)

            # partial ks[b, n] = sum_k h[b, k] * W2s[k, n] ; W2s streamed
            for g0, gw in GROUPS:
                nts = [(t, min(512, gw - t)) for t in range(0, gw, 512)]
                pts = [psK.tile([B, 512], F32, tag="pk", name=f"pk_{g0}_{t0}")
                       for t0, _ in nts]
                for c in range(NCH):
                    kc = 128 if c < NCH - 1 else KP - 128 * (NCH - 1)
                    w2t = w2pool.tile([128, 2048], F32, tag="w2t",
                                      name=f"w2t_{g0}_{c}")
                    nc.sync.dma_start(out=w2t[0:kc, 0:gw],
                                      in_=w2s[128 * c:128 * c + kc, g0:g0 + gw])
                    for ti, (t0, tw) in enumerate(nts):
                        nc.tensor.matmul(
                            pts[ti][:, 0:tw],
                            lhsT=hT_sb[0:kc, c * B:(c + 1) * B],
                            rhs=w2t[0:kc, t0:t0 + tw],
                            start=(c == 0), stop=(c == NCH - 1))
                ks_sb = kspool.tile([B, 2048], F32, tag="ks_sb", name=f"ks_{g0}")
                for ti, (t0, tw) in enumerate(nts):
                    nc.scalar.activation(out=ks_sb[:, t0:t0 + tw],
                                         in_=pts[ti][:, 0:tw], func=AF.Copy)
                nc.sync.dma_start(out=ks_part[:, g0:g0 + gw], in_=ks_sb[:, 0:gw])

        # ------------- ReduceScatter: core i gets ks rows 4i..4i+3 --------
        nc.gpsimd.collective_compute(
            "ReduceScatter", mybir.AluOpType.add,
            replica_groups=[list(range(N_CORES))],
            ins=[ks_part[:, :].opt()], outs=[ks_own[:, :].opt()])

        # ---------------- Phase B: per-sample 1x1 convs -------------------
        with tc.tile_pool(name="wts", bufs=1) as wts, \
             tc.tile_pool(name="ps1", bufs=2, space="PSUM") as ps1p, \
             tc.tile_pool(name="ps2", bufs=2, space="PSUM") as ps2p, \
             tc.tile_pool(name="ps3", bufs=2, space="PSUM") as ps3p, \
             tc.tile_pool(name="xin", bufs=3) as xinp, \
             tc.tile_pool(name="ys", bufs=3) as ysp, \
             tc.tile_pool(name="outp", bufs=3) as outp:
            pair_wts = []
            for p in range(2):
                sa = 2 * p
                L1 = wts.tile([XC, 128], F32, name=f"L1_{p}")
                L2 = wts.tile([128, 128], F32, name=f"L2_{p}")
                L3 = wts.tile([128, FH], F32, name=f"L3_{p}")
                L4 = wts.tile([XC, FH], F32, name=f"L4_{p}")
                bmid = wts.tile([128, 1], F32, name=f"bm_{p}")
                for t in (L1, L2, L3, L4):
                    nc.vector.memset(t, 0.0)
                for s in range(2):
                    row = sa + s
                    # conv kernels, already transposed to lhsT layout by the
                    # host-side W2 column permutation
                    nc.gpsimd.dma_start(
                        out=L1[32 * s:32 * s + 32, 64 * s:64 * s + 64],
                        in_=ks_own[row, OFF_IN:OFF_IN + K_IN]
                        .rearrange("(i o) -> i o", i=FIN))
                    nc.gpsimd.dma_start(
                        out=L2[64 * s:64 * s + 64, 64 * s:64 * s + 64],
                        in_=ks_own[row, OFF_MID:OFF_MID + K_MID]
                        .rearrange("(i o) -> i o", i=FH))
                    nc.gpsimd.dma_start(
                        out=L3[64 * s:64 * s + 64, 32 * s:32 * s + 32],
                        in_=ks_own[row, OFF_OUT:OFF_OUT + K_OUT]
                        .rearrange("(i o) -> i o", i=FH))
                    nc.gpsimd.dma_start(
                        out=L4[32 * s:32 * s + 32, 32 * s:32 * s + 32],
                        in_=ks_own[row, OFF_SHC:OFF_SHC + K_SH]
                        .rearrange("(i o) -> i o", i=FIN))
                    # bias rows, consumed via the ones-channels of xs
                    nc.gpsimd.dma_start(out=L1[64 + s:65 + s, 64 * s:64 * s + 64],
                                        in_=ks_own[row, OB_IN:OB_IN + FH])
                    nc.gpsimd.dma_start(out=L4[64 + s:65 + s, 32 * s:32 * s + 32],
                                        in_=ks_own[row, OB_SH:OB_SH + FOUT])
                    nc.gpsimd.dma_start(out=L4[66:67, 32 * s:32 * s + 32],
                                        in_=ks_own[row, OB_OUT:OB_OUT + FOUT])
                nc.gpsimd.dma_start(out=bmid[:, 0:1],
                                    in_=ks_own[sa:sa + 2, OB_MID:OB_MID + FH])
                pair_wts.append((L1, L2, L3, L4, bmid))

            spl = 512 - DVE_COLS
            for p in range(2):
                L1, L2, L3, L4, bmid = pair_wts[p]
                for j in range(HW // 512):
                    c0 = 512 * j
                    x_t = xinp.tile([XC, 512], F32, tag="x", name=f"x_{p}_{j}")
                    nc.sync.dma_start(out=x_t, in_=xs[p, :, c0:c0 + 512])

                    p1 = ps1p.tile([128, 512], F32, tag="p1", name=f"p1_{p}_{j}")
                    nc.tensor.matmul(p1, lhsT=L1, rhs=x_t, start=True, stop=True)
                    y1 = ysp.tile([128, 512], F32, tag="y1", name=f"y1_{p}_{j}")
                    if DVE_COLS > 0:
                        nc.scalar.activation(out=y1[:, 0:spl], in_=p1[:, 0:spl],
                                             func=AF.Lrelu, bias=0.0, scale=1.0,
                                             alpha=0.01)
                        nc.vector.scalar_tensor_tensor(
                            out=y1[:, spl:512], in0=p1[:, spl:512], scalar=0.01,
                            in1=p1[:, spl:512],
                            op0=mybir.AluOpType.mult, op1=mybir.AluOpType.max)
                    else:
                        nc.scalar.activation(out=y1, in_=p1, func=AF.Lrelu,
                                             bias=0.0, scale=1.0, alpha=0.01)

                    p2 = ps2p.tile([128, 512], F32, tag="p2", name=f"p2_{p}_{j}")
                    nc.tensor.matmul(p2, lhsT=L2, rhs=y1, start=True, stop=True)
                    y2 = ysp.tile([128, 512], F32, tag="y2", name=f"y2_{p}_{j}")
                    nc.scalar.activation(out=y2, in_=p2, func=AF.Lrelu,
                                         bias=bmid[:, 0:1], scale=1.0, alpha=0.01)

                    p3 = ps3p.tile([FH, 512], F32, tag="p3", name=f"p3_{p}_{j}")
                    nc.tensor.matmul(p3, lhsT=L3, rhs=y2, start=True, stop=False)
                    nc.tensor.matmul(p3, lhsT=L4, rhs=x_t, start=False, stop=True)
                    o_sb = outp.tile([FH, 512], F32, tag="o", name=f"o_{p}_{j}")
                    nc.vector.tensor_copy(o_sb, p3)
                    nc.sync.dma_start(out=out[p, :, c0:c0 + 512], in_=o_sb)

    nc.compile()
    return nc


def _seg_perm(rows, cols):
    # new position (c, r) holds old flat index r*cols + c
    return np.arange(rows * cols).reshape(rows, cols).T.ravel()


def _perm():
    # permutation of ks columns so each conv kernel arrives transposed
    return np.concatenate([
        OFF_IN + _seg_perm(FH, FIN),
        OFF_MID + _seg_perm(FH, FH),
        OFF_OUT + _seg_perm(FOUT, FH),
        OFF_SHC + _seg_perm(FOUT, FIN),
        np.arange(OFF_B, K_TOT),
    ])


def _prep_in_maps(x, lat, W1, b1, W2, b2):
    x = np.ascontiguousarray(x, np.float32)
    lat = np.ascontiguousarray(lat, np.float32)
    W1 = np.ascontiguousarray(W1, np.float32)
    b1 = np.asarray(b1, np.float32)
    W2 = np.asarray(W2, np.float32)
    b2 = np.asarray(b2, np.float32)

    perm = _perm()
    W2p = W2[:, perm]
    b2p = b2[perm]
    latT = np.ascontiguousarray(lat.T)
    xr = x.reshape(B, FIN, HW)

    in_maps = []
    for i in range(N_CORES):
        sh = slice(i * SHARD, (i + 1) * SHARD)
        w1p = np.zeros((LAT, KPAD), np.float32)
        w1p[:, :SHARD] = W1[:, sh]
        b1p = np.zeros((KPAD,), np.float32)
        b1p[:SHARD] = b1[sh]
        b1p[SHARD] = 1.0  # the "ones" h-slot that carries b2
        w2a = np.zeros((KP, K_TOT), np.float32)
        w2a[:SHARD] = W2p[sh]
        if i == 0:
            w2a[SHARD] = b2p
        xsi = np.ones((2, XC, HW), np.float32)
        for p in range(2):
            xsi[p, 0:FIN] = xr[4 * i + 2 * p]
            xsi[p, FIN:2 * FIN] = xr[4 * i + 2 * p + 1]
        in_maps.append({
            "xs": xsi,
            "latT": latT,
            "w1s": w1p,
            "b1s": np.ascontiguousarray(b1p.reshape(NCH, 128).T),
            "w2s": w2a,
        })
    return in_maps


def _run(in_maps, **kwargs):
    if "nc" not in _CACHE:
        _CACHE["nc"] = _build()
    return run_bass_kernel_spmd(_CACHE["nc"], in_maps,
                                core_ids=list(range(N_CORES)), **kwargs)


def _assemble(results):
    parts = [r["out"].reshape(BPC, FOUT, H, W) for r in results]
    return np.ascontiguousarray(np.concatenate(parts, axis=0))


def kernel(x, lat, W1, b1, W2, b2):
    in_maps = _prep_in_maps(x, lat, W1, b1, W2, b2)
    res = _run(in_maps)
    return _assemble(res.results)


# revision 5
# speedup vs baseline: 1.1911x; 1.1911x over previous
"""Trainium2 Bass kernel for nn_DynaResidualBlock (hypernetwork residual block).

Reference computation (B=32, LAT=256, FIN=FOUT=32, FH=64, H=W=128):
    h  = lat @ W1 + b1                       # [B, 9408]
    ks = h @ W2 + b2                         # [B, 9408]  (W2 is 9408x9408 = 354 MB)
    per-sample 1x1 convs with kernels/biases sliced out of ks:
    x_s = k_short(x) ; y = k_out(lrelu(k_mid(lrelu(k_in(x))))) + x_s

Sharding over 8 cores:
  - hypernet contraction dim (9408) split 1176-per-core: core i holds
    W1[:, shard_i] and W2[shard_i, :] and computes a partial ks for ALL
    32 samples; a ReduceScatter then hands core i the summed ks rows for
    its own 4 samples.
  - conv phase is data-parallel: core i processes samples 4i..4i+3, packed
    as 2 sample-pairs with block-diagonal weight matrices so each 1x1 conv
    over a 512-pixel tile is a single PE matmul.

Host-side layout tricks (pure data reformatting, no reference FLOPs):
  - W2's columns are permuted so each generated conv kernel lands in
    SBUF already transposed into the PE's lhsT layout (no on-chip
    transposes needed).
  - x gets 3 constant "ones" channels per pair so conv biases ride in as
    extra matmul rows (b_in, b_short, b_out fold into the matmuls).
  - lat is passed pre-transposed; b2 rides as an extra W2 row on core 0.
"""

import contextlib

import numpy as np

import concourse.bacc as bacc
import concourse.mybir as mybir
import concourse.tile as tile
from concourse.bass_utils import run_bass_kernel_spmd

N_CORES = 8
B, LAT, FIN, FOUT, FH, H, W = 32, 256, 32, 32, 64, 128, 128
HW = H * W
K_IN, K_MID, K_OUT, K_SH = FH * FIN, FH * FH, FOUT * FH, FOUT * FIN
K_TOT = K_IN + K_MID + K_OUT + K_SH + FH + FH + FOUT + FOUT  # 9408
SHARD = K_TOT // N_CORES  # 1176 hypernet columns per core
KP = SHARD + 1            # + one bias row (b2, on core 0 only)
KPAD = 1280               # h length padded to 10 chunks of 128
NCH = KPAD // 128         # 10
BPC = B // N_CORES        # 4 samples per core
XC = 2 * FIN + 3          # 67 = 2x32 x-channels + 3 ones-channels per pair
F32 = mybir.dt.float32
F32R = mybir.dt.float32r  # FP22 multiplies at full PE rate, fp32 accumulate


def _r(ap):
    return ap.bitcast(F32R)

OFF_IN, OFF_MID = 0, K_IN
OFF_OUT, OFF_SHC = K_IN + K_MID, K_IN + K_MID + K_OUT
OFF_B = OFF_SHC + K_SH  # 9216: b_in 64 | b_mid 64 | b_out 32 | b_short 32
OB_IN, OB_MID = OFF_B, OFF_B + FH
OB_OUT, OB_SH = OFF_B + 2 * FH, OFF_B + 2 * FH + FOUT

# ks column groups for phase A (align with conv-weight segment boundaries)
GROUPS = [(0, 2048), (2048, 2048), (4096, 2048), (6144, 2048), (8192, 1216)]

# columns of y1's leaky-relu computed on DVE (rest on ACT); 0 = all on ACT.
# NOTE: the DVE path needs two PSUM reads in one op, which walrus rejects —
# keep 0 unless restructured.
DVE_COLS = 0

_CACHE: dict = {}


def _build():
    nc = bacc.Bacc("TRN2", target_bir_lowering=False, num_devices=N_CORES)
    AF = mybir.ActivationFunctionType

    xs = nc.dram_tensor("xs", [2, XC, HW], F32, kind="ExternalInput")
    latT = nc.dram_tensor("latT", [LAT, B], F32, kind="ExternalInput")
    w1s = nc.dram_tensor("w1s", [LAT, KPAD], F32, kind="ExternalInput")
    b1s = nc.dram_tensor("b1s", [128, NCH], F32, kind="ExternalInput")
    w2s = nc.dram_tensor("w2s", [KP, K_TOT], F32, kind="ExternalInput")
    out = nc.dram_tensor("out", [2, 2 * FOUT, HW], F32, kind="ExternalOutput")

    zeros_dram = nc.inline_tensor(np.zeros((128, 128), np.float32), name="zconst")
    ks_part = nc.dram_tensor("ks_part", [B, K_TOT], F32)
    ks_own = nc.dram_tensor("ks_own", [BPC, K_TOT], F32)

    with tile.TileContext(nc) as tc, contextlib.ExitStack() as ctx:
        sing = ctx.enter_context(tc.tile_pool(name="sing", bufs=1))

        # ---------------- Phase A: hypernet (partial ks for all B) --------
        with tc.tile_pool(name="psA", bufs=2, space="PSUM") as psA, \
             tc.tile_pool(name="psK", bufs=6, space="PSUM") as psK, \
             tc.tile_pool(name="w2pool", bufs=4) as w2pool, \
             tc.tile_pool(name="kspool", bufs=2) as kspool:
            latT_sb = sing.tile([128, 2 * B], F32R)
            w1_sb = sing.tile([128, 2 * KPAD], F32R)
            b1_sb = sing.tile([128, NCH], F32)
            hT_sb = sing.tile([128, NCH * B], F32R)
            for l in range(2):
                nc.sync.dma_start(out=latT_sb[:, l * B:(l + 1) * B],
                                  in_=_r(latT[128 * l:128 * (l + 1), :]))
                nc.sync.dma_start(out=w1_sb[:, l * KPAD:(l + 1) * KPAD],
                                  in_=_r(w1s[128 * l:128 * (l + 1), :]))
            nc.sync.dma_start(out=b1_sb[:, :], in_=b1s[:, :])

            # hT[kcol, b] = sum_l W1[l, kcol] * lat[b, l]  (+ b1[kcol])
            for c in range(NCH):
                ph = psA.tile([128, B], F32, tag="ph", name=f"ph{c}")
                for l in range(2):
                    nc.tensor.matmul(
                        ph,
                        lhsT=w1_sb[:, l * KPAD + 128 * c: l * KPAD + 128 * c + 128],
                        rhs=latT_sb[:, l * B:(l + 1) * B],
                        start=(l == 0), stop=(l == 1))
                nc.scalar.activation(out=hT_sb[:, c * B:(c + 1) * B], in_=ph,
                                     func=AF.Identity, bias=b1_sb[:, c:c + 1],
                                     scale=1.0)

            # partial ks[b, n] = sum_k h[b, k] * W2s[k, n] ; W2s streamed
            for g0, gw in GROUPS:
                nts = [(t, min(512, gw - t)) for t in range(0, gw, 512)]
                pts = [psK.tile([B, 512], F32, tag="pk", name=f"pk_{g0}_{t0}")
                       for t0, _ in nts]
                for c in range(NCH):
                    kc = 128 if c < NCH - 1 else KP - 128 * (NCH - 1)
                    w2t = w2pool.tile([128, 2048], F32R, tag="w2t",
                                      name=f"w2t_{g0}_{c}")
                    nc.sync.dma_start(out=w2t[0:kc, 0:gw],
                                      in_=_r(w2s[128 * c:128 * c + kc, g0:g0 + gw]))
                    for ti, (t0, tw) in enumerate(nts):
                        nc.tensor.matmul(
                            pts[ti][:, 0:tw],
                            lhsT=hT_sb[0:kc, c * B:(c + 1) * B],
                            rhs=w2t[0:kc, t0:t0 + tw],
                            start=(c == 0), stop=(c == NCH - 1))
                ks_sb = kspool.tile([B, 2048], F32, tag="ks_sb", name=f"ks_{g0}")
                for ti, (t0, tw) in enumerate(nts):
                    nc.scalar.activation(out=ks_sb[:, t0:t0 + tw],
                                         in_=pts[ti][:, 0:tw], func=AF.Copy)
                nc.sync.dma_start(out=ks_part[:, g0:g0 + gw], in_=ks_sb[:, 0:gw])

        # ------------- ReduceScatter: core i gets ks rows 4i..4i+3 --------
        nc.gpsimd.collective_compute(
            "ReduceScatter", mybir.AluOpType.add,
            replica_groups=[list(range(N_CORES))],
            ins=[ks_part[:, :].opt()], outs=[ks_own[:, :].opt()])

        # ---------------- Phase B: per-sample 1x1 convs -------------------
        with tc.tile_pool(name="wts", bufs=1) as wts, \
             tc.tile_pool(name="ps1", bufs=3, space="PSUM") as ps1p, \
             tc.tile_pool(name="ps2", bufs=3, space="PSUM") as ps2p, \
             tc.tile_pool(name="ps3", bufs=2, space="PSUM") as ps3p, \
             tc.tile_pool(name="xin", bufs=4) as xinp, \
             tc.tile_pool(name="ys", bufs=3) as ysp, \
             tc.tile_pool(name="outp", bufs=3) as outp:
            pair_wts = []
            for p in range(2):
                sa = 2 * p
                L1 = wts.tile([XC, 128], F32R, name=f"L1_{p}")
                L2 = wts.tile([128, 128], F32R, name=f"L2_{p}")
                L3 = wts.tile([128, FH], F32R, name=f"L3_{p}")
                L4 = wts.tile([XC, FH], F32R, name=f"L4_{p}")
                bmid = wts.tile([128, 1], F32, name=f"bm_{p}")
                for t in (L1, L2, L3, L4):
                    pp, ff = t.shape
                    nc.gpsimd.dma_start(out=t, in_=_r(zeros_dram[0:pp, 0:ff]))
                for s in range(2):
                    row = sa + s
                    # conv kernels, already transposed to lhsT layout by the
                    # host-side W2 column permutation
                    nc.gpsimd.dma_start(
                        out=L1[32 * s:32 * s + 32, 64 * s:64 * s + 64],
                        in_=_r(ks_own[row, OFF_IN:OFF_IN + K_IN]
                               .rearrange("(i o) -> i o", i=FIN)))
                    nc.gpsimd.dma_start(
                        out=L2[64 * s:64 * s + 64, 64 * s:64 * s + 64],
                        in_=_r(ks_own[row, OFF_MID:OFF_MID + K_MID]
                               .rearrange("(i o) -> i o", i=FH)))
                    nc.gpsimd.dma_start(
                        out=L3[64 * s:64 * s + 64, 32 * s:32 * s + 32],
                        in_=_r(ks_own[row, OFF_OUT:OFF_OUT + K_OUT]
                               .rearrange("(i o) -> i o", i=FH)))
                    nc.gpsimd.dma_start(
                        out=L4[32 * s:32 * s + 32, 32 * s:32 * s + 32],
                        in_=_r(ks_own[row, OFF_SHC:OFF_SHC + K_SH]
                               .rearrange("(i o) -> i o", i=FIN)))
                    # bias rows, consumed via the ones-channels of xs
                    nc.gpsimd.dma_start(out=L1[64 + s:65 + s, 64 * s:64 * s + 64],
                                        in_=_r(ks_own[row, OB_IN:OB_IN + FH]))
                    nc.gpsimd.dma_start(out=L4[64 + s:65 + s, 32 * s:32 * s + 32],
                                        in_=_r(ks_own[row, OB_SH:OB_SH + FOUT]))
                    nc.gpsimd.dma_start(out=L4[66:67, 32 * s:32 * s + 32],
                                        in_=_r(ks_own[row, OB_OUT:OB_OUT + FOUT]))
                nc.gpsimd.dma_start(out=bmid[:, 0:1],
                                    in_=ks_own[sa:sa + 2, OB_MID:OB_MID + FH])
                pair_wts.append((L1, L2, L3, L4, bmid))

            spl = 512 - DVE_COLS
            for p in range(2):
                L1, L2, L3, L4, bmid = pair_wts[p]
                for j in range(HW // 512):
                    c0 = 512 * j
                    x_t = xinp.tile([XC, 512], F32R, tag="x", name=f"x_{p}_{j}")
                    nc.sync.dma_start(out=x_t, in_=_r(xs[p, :, c0:c0 + 512]))

                    p1 = ps1p.tile([128, 512], F32, tag="p1", name=f"p1_{p}_{j}")
                    nc.tensor.matmul(p1, lhsT=L1, rhs=x_t, start=True, stop=True)
                    y1 = ysp.tile([128, 512], F32R, tag="y1", name=f"y1_{p}_{j}")
                    if DVE_COLS > 0:
                        nc.scalar.activation(out=y1[:, 0:spl], in_=p1[:, 0:spl],
                                             func=AF.Lrelu, bias=0.0, scale=1.0,
                                             alpha=0.01)
                        nc.vector.scalar_tensor_tensor(
                            out=y1[:, spl:512], in0=p1[:, spl:512], scalar=0.01,
                            in1=p1[:, spl:512],
                            op0=mybir.AluOpType.mult, op1=mybir.AluOpType.max)
                    else:
                        nc.scalar.activation(out=y1, in_=p1, func=AF.Lrelu,
                                             bias=0.0, scale=1.0, alpha=0.01)

                    p2 = ps2p.tile([128, 512], F32, tag="p2", name=f"p2_{p}_{j}")
                    nc.tensor.matmul(p2, lhsT=L2, rhs=y1, start=True, stop=True)
                    y2 = ysp.tile([128, 512], F32R, tag="y2", name=f"y2_{p}_{j}")
                    nc.scalar.activation(out=y2, in_=p2, func=AF.Lrelu,
                                         bias=bmid[:, 0:1], scale=1.0, alpha=0.01)

                    p3 = ps3p.tile([FH, 512], F32, tag="p3", name=f"p3_{p}_{j}")
                    nc.tensor.matmul(p3, lhsT=L3, rhs=y2, start=True, stop=False)
                    nc.tensor.matmul(p3, lhsT=L4, rhs=x_t, start=False, stop=True)
                    o_sb = outp.tile([FH, 512], F32, tag="o", name=f"o_{p}_{j}")
                    nc.vector.tensor_copy(o_sb, p3)
                    nc.sync.dma_start(out=out[p, :, c0:c0 + 512], in_=o_sb)

    nc.compile()
    return nc


def _seg_perm(rows, cols):
    # new position (c, r) holds old flat index r*cols + c
    return np.arange(rows * cols).reshape(rows, cols).T.ravel()


def _perm():
    # permutation of ks columns so each conv kernel arrives transposed
    return np.concatenate([
        OFF_IN + _seg_perm(FH, FIN),
        OFF_MID + _seg_perm(FH, FH),
        OFF_OUT + _seg_perm(FOUT, FH),
        OFF_SHC + _seg_perm(FOUT, FIN),
        np.arange(OFF_B, K_TOT),
    ])


def _prep_in_maps(x, lat, W1, b1, W2, b2):
    x = np.ascontiguousarray(x, np.float32)
    lat = np.ascontiguousarray(lat, np.float32)
    W1 = np.ascontiguousarray(W1, np.float32)
    b1 = np.asarray(b1, np.float32)
    W2 = np.asarray(W2, np.float32)
    b2 = np.asarray(b2, np.float32)

    perm = _perm()
    W2p = W2[:, perm]
    b2p = b2[perm]
    latT = np.ascontiguousarray(lat.T)
    xr = x.reshape(B, FIN, HW)

    in_maps = []
    for i in range(N_CORES):
        sh = slice(i * SHARD, (i + 1) * SHARD)
        w1p = np.zeros((LAT, KPAD), np.float32)
        w1p[:, :SHARD] = W1[:, sh]
        b1p = np.zeros((KPAD,), np.float32)
        b1p[:SHARD] = b1[sh]
        b1p[SHARD] = 1.0  # the "ones" h-slot that carries b2
        w2a = np.zeros((KP, K_TOT), np.float32)
        w2a[:SHARD] = W2p[sh]
        if i == 0:
            w2a[SHARD] = b2p
        xsi = np.ones((2, XC, HW), np.float32)
        for p in range(2):
            xsi[p, 0:FIN] = xr[4 * i + 2 * p]
            xsi[p, FIN:2 * FIN] = xr[4 * i + 2 * p + 1]
        in_maps.append({
            "xs": xsi,
            "latT": latT,
            "w1s": w1p,
            "b1s": np.ascontiguousarray(b1p.reshape(NCH, 128).T),
            "w2s": w2a,
        })
    return in_maps


def _run(in_maps, **kwargs):
    if "nc" not in _CACHE:
        _CACHE["nc"] = _build()
    return run_bass_kernel_spmd(_CACHE["nc"], in_maps,
                                core_ids=list(range(N_CORES)), **kwargs)


def _assemble(results):
    parts = [r["out"].reshape(BPC, FOUT, H, W) for r in results]
    return np.ascontiguousarray(np.concatenate(parts, axis=0))


def kernel(x, lat, W1, b1, W2, b2):
    in_maps = _prep_in_maps(x, lat, W1, b1, W2, b2)
    res = _run(in_maps)
    return _assemble(res.results)


# revision 6
# speedup vs baseline: 1.2663x; 1.0631x over previous
"""Trainium2 Bass kernel for nn_DynaResidualBlock (hypernetwork residual block).

Reference computation (B=32, LAT=256, FIN=FOUT=32, FH=64, H=W=128):
    h  = lat @ W1 + b1                       # [B, 9408]
    ks = h @ W2 + b2                         # [B, 9408]  (W2 is 9408x9408 = 354 MB)
    per-sample 1x1 convs with kernels/biases sliced out of ks:
    x_s = k_short(x) ; y = k_out(lrelu(k_mid(lrelu(k_in(x))))) + x_s

Sharding over 8 cores:
  - hypernet contraction dim (9408) split 1176-per-core: core i holds
    W1[:, shard_i] and W2[shard_i, :] and computes a partial ks for ALL
    32 samples; per-segment ReduceScatters (overlapped with the W2
    streaming) hand core i the summed ks rows for its own 4 samples.
  - conv phase is data-parallel: core i processes samples 4i..4i+3, packed
    as 2 sample-pairs with block-diagonal weight matrices so each 1x1 conv
    over a 512-pixel tile is a single PE matmul.

Implementation notes:
  - All matmuls run as float32r (FP22 multiply, fp32 accumulate): single
    PE pass at full rate vs fp32's two half-rate passes.
  - fp32r matmuls do not register as PE activity for the HAM clock gate,
    which would leave the PE throttled at 1.2 GHz; tiny bf16 "warmer"
    matmuls are interleaved to keep the 2.4 GHz clock.
  - W2's columns are permuted host-side so each generated conv kernel
    lands in SBUF already transposed into the PE's lhsT layout.
  - x gets 3 constant "ones" channels per pair so conv biases ride in as
    extra matmul rows (b_in, b_short, b_out fold into the matmuls).
  - lat is passed pre-transposed; b2 rides as an extra W2 row on core 0.
"""

import contextlib

import numpy as np

import concourse.bacc as bacc
import concourse.mybir as mybir
import concourse.tile as tile
from concourse.bass_utils import run_bass_kernel_spmd

N_CORES = 8
B, LAT, FIN, FOUT, FH, H, W = 32, 256, 32, 32, 64, 128, 128
HW = H * W
K_IN, K_MID, K_OUT, K_SH = FH * FIN, FH * FH, FOUT * FH, FOUT * FIN
K_TOT = K_IN + K_MID + K_OUT + K_SH + FH + FH + FOUT + FOUT  # 9408
SHARD = K_TOT // N_CORES  # 1176 hypernet columns per core
KP = SHARD + 1            # + one bias row (b2, on core 0 only)
KPAD = 1280               # h length padded to 10 chunks of 128
NCH = KPAD // 128         # 10
BPC = B // N_CORES        # 4 samples per core
XC = 2 * FIN + 3          # 67 = 2x32 x-channels + 3 ones-channels per pair
F32 = mybir.dt.float32
F32R = mybir.dt.float32r  # FP22 multiplies at full PE rate, fp32 accumulate
BF16 = mybir.dt.bfloat16


def _r(ap):
    return ap.bitcast(F32R)


OFF_IN, OFF_MID = 0, K_IN
OFF_OUT, OFF_SHC = K_IN + K_MID, K_IN + K_MID + K_OUT
OFF_B = OFF_SHC + K_SH  # 9216: b_in 64 | b_mid 64 | b_out 32 | b_short 32

# phase-A ks column groups (512-multiples; last is the 1216 tail)
GROUPS = [(0, 2048), (2048, 2048), (4096, 2048), (6144, 2048), (8192, 1216)]
# ReduceScatter splits, aligned to conv-weight segments:
#   rs0 = k_in | rs1 = k_mid | rs2 = k_out | rs3 = k_short + biases
RS_SPEC = [(0, K_IN), (OFF_MID, K_MID), (OFF_OUT, K_OUT), (OFF_SHC, K_SH + 192)]
# group index -> rs index fired after that group's store completes
RS_AFTER_GROUP = {0: 0, 2: 1, 3: 2, 4: 3}
# bias offsets inside rs3's output
B_IN3, B_MID3, B_OUT3, B_SH3 = K_SH, K_SH + FH, K_SH + 2 * FH, K_SH + 2 * FH + FOUT

_CACHE: dict = {}


def _build():
    nc = bacc.Bacc("TRN2", target_bir_lowering=False, num_devices=N_CORES)
    AF = mybir.ActivationFunctionType

    xs = nc.dram_tensor("xs", [2, XC, HW], F32, kind="ExternalInput")
    latT = nc.dram_tensor("latT", [LAT, B], F32, kind="ExternalInput")
    w1s = nc.dram_tensor("w1s", [LAT, KPAD], F32, kind="ExternalInput")
    b1s = nc.dram_tensor("b1s", [128, NCH], F32, kind="ExternalInput")
    w2s = nc.dram_tensor("w2s", [KP, K_TOT], F32, kind="ExternalInput")
    out = nc.dram_tensor("out", [2, 2 * FOUT, HW], F32, kind="ExternalOutput")

    zeros_dram = nc.inline_tensor(np.zeros((128, 128), np.float32), name="zconst")
    ks_part = [nc.dram_tensor(f"ks_part{g}", [B, w], F32)
               for g, (_, w) in enumerate(RS_SPEC)]
    ks_own = [nc.dram_tensor(f"ks_own{g}", [BPC, w], F32)
              for g, (_, w) in enumerate(RS_SPEC)]

    with tile.TileContext(nc) as tc, contextlib.ExitStack() as ctx:
        sing = ctx.enter_context(tc.tile_pool(name="sing", bufs=1))
        psD = ctx.enter_context(tc.tile_pool(name="psD", bufs=1, space="PSUM"))

        # tiny bf16 operands for the HAM-warmer matmuls
        wk_w = sing.tile([1, 2], BF16)
        wk_r = sing.tile([1, 64], BF16)
        nc.gpsimd.memset(wk_w, 0.0)
        nc.gpsimd.memset(wk_r, 0.0)

        def warm(name):
            pd = psD.tile([1, 64], F32, tag="pd", name=name)
            nc.tensor.matmul(pd, lhsT=wk_w[0:1, 0:1], rhs=wk_r,
                             start=True, stop=True)

        # ---------------- Phase A: hypernet (partial ks for all B) --------
        with tc.tile_pool(name="psA", bufs=1, space="PSUM") as psA, \
             tc.tile_pool(name="psK", bufs=6, space="PSUM") as psK, \
             tc.tile_pool(name="w2pool", bufs=4) as w2pool, \
             tc.tile_pool(name="kspool", bufs=2) as kspool:
            latT_sb = sing.tile([128, 2 * B], F32R)
            w1_sb = sing.tile([128, 2 * KPAD], F32R)
            b1_sb = sing.tile([128, NCH], F32)
            hT_sb = sing.tile([128, NCH * B], F32R)
            for l in range(2):
                nc.sync.dma_start(out=latT_sb[:, l * B:(l + 1) * B],
                                  in_=_r(latT[128 * l:128 * (l + 1), :]))
                nc.sync.dma_start(out=w1_sb[:, l * KPAD:(l + 1) * KPAD],
                                  in_=_r(w1s[128 * l:128 * (l + 1), :]))
            nc.sync.dma_start(out=b1_sb[:, :], in_=b1s[:, :])

            # hT[kcol, b] = sum_l W1[l, kcol] * lat[b, l]  (+ b1[kcol])
            for c in range(NCH):
                ph = psA.tile([128, B], F32, tag="ph", name=f"ph{c}")
                for l in range(2):
                    nc.tensor.matmul(
                        ph,
                        lhsT=w1_sb[:, l * KPAD + 128 * c: l * KPAD + 128 * c + 128],
                        rhs=latT_sb[:, l * B:(l + 1) * B],
                        start=(l == 0), stop=(l == 1))
                nc.scalar.activation(out=hT_sb[:, c * B:(c + 1) * B], in_=ph,
                                     func=AF.Identity, bias=b1_sb[:, c:c + 1],
                                     scale=1.0)

            # partial ks[b, n] = sum_k h[b, k] * W2s[k, n] ; W2s streamed
            for g, (g0, gw) in enumerate(GROUPS):
                nts = [(t, min(512, gw - t)) for t in range(0, gw, 512)]
                pts = [psK.tile([B, 512], F32, tag="pk", name=f"pk_{g0}_{t0}")
                       for t0, _ in nts]
                for c in range(NCH):
                    kc = 128 if c < NCH - 1 else KP - 128 * (NCH - 1)
                    w2t = w2pool.tile([128, 2048], F32R, tag="w2t",
                                      name=f"w2t_{g0}_{c}")
                    nc.sync.dma_start(out=w2t[0:kc, 0:gw],
                                      in_=_r(w2s[128 * c:128 * c + kc, g0:g0 + gw]))
                    for ti, (t0, tw) in enumerate(nts):
                        nc.tensor.matmul(
                            pts[ti][:, 0:tw],
                            lhsT=hT_sb[0:kc, c * B:(c + 1) * B],
                            rhs=w2t[0:kc, t0:t0 + tw],
                            start=(c == 0), stop=(c == NCH - 1))
                    warm(f"wa_{g}_{c}")
                ks_sb = kspool.tile([B, 2048], F32, tag="ks_sb", name=f"ks_{g0}")
                for ti, (t0, tw) in enumerate(nts):
                    nc.scalar.activation(out=ks_sb[:, t0:t0 + tw],
                                         in_=pts[ti][:, 0:tw], func=AF.Copy)
                # store into the right per-RS partial tensor
                rs_i = next(i for i, (r0, rw) in enumerate(RS_SPEC)
                            if r0 <= g0 < r0 + rw)
                r0, rw = RS_SPEC[rs_i]
                nc.sync.dma_start(out=ks_part[rs_i][:, g0 - r0:g0 - r0 + gw],
                                  in_=ks_sb[:, 0:gw])
                if g in RS_AFTER_GROUP:
                    i = RS_AFTER_GROUP[g]
                    nc.gpsimd.collective_compute(
                        "ReduceScatter", mybir.AluOpType.add,
                        replica_groups=[list(range(N_CORES))],
                        ins=[ks_part[i][:, :].opt()],
                        outs=[ks_own[i][:, :].opt()])

        ko0, ko1, ko2, ko3 = ks_own

        # ---------------- Phase B: per-sample 1x1 convs -------------------
        with tc.tile_pool(name="wts", bufs=1) as wts, \
             tc.tile_pool(name="ps1", bufs=2, space="PSUM") as ps1p, \
             tc.tile_pool(name="ps2", bufs=3, space="PSUM") as ps2p, \
             tc.tile_pool(name="ps3", bufs=2, space="PSUM") as ps3p, \
             tc.tile_pool(name="xin", bufs=3) as xinp, \
             tc.tile_pool(name="ys", bufs=3) as ysp, \
             tc.tile_pool(name="outp", bufs=3) as outp:
            pair_wts = []
            for p in range(2):
                sa = 2 * p
                L1 = wts.tile([XC, 128], F32R, name=f"L1_{p}")
                L2 = wts.tile([128, 128], F32R, name=f"L2_{p}")
                L3 = wts.tile([128, FH], F32R, name=f"L3_{p}")
                L4 = wts.tile([XC, FH], F32R, name=f"L4_{p}")
                bmid = wts.tile([128, 1], F32, name=f"bm_{p}")
                for t in (L1, L2, L3, L4):
                    pp, ff = t.shape
                    nc.gpsimd.dma_start(out=t, in_=_r(zeros_dram[0:pp, 0:ff]))
                for s in range(2):
                    row = sa + s
                    # conv kernels, already transposed to lhsT layout by the
                    # host-side W2 column permutation
                    nc.gpsimd.dma_start(
                        out=L1[32 * s:32 * s + 32, 64 * s:64 * s + 64],
                        in_=_r(ko0[row, :].rearrange("(i o) -> i o", i=FIN)))
                    nc.gpsimd.dma_start(
                        out=L2[64 * s:64 * s + 64, 64 * s:64 * s + 64],
                        in_=_r(ko1[row, :].rearrange("(i o) -> i o", i=FH)))
                    nc.gpsimd.dma_start(
                        out=L3[64 * s:64 * s + 64, 32 * s:32 * s + 32],
                        in_=_r(ko2[row, :].rearrange("(i o) -> i o", i=FH)))
                    nc.gpsimd.dma_start(
                        out=L4[32 * s:32 * s + 32, 32 * s:32 * s + 32],
                        in_=_r(ko3[row, 0:K_SH].rearrange("(i o) -> i o", i=FIN)))
                    # bias rows, consumed via the ones-channels of xs
                    nc.gpsimd.dma_start(out=L1[64 + s:65 + s, 64 * s:64 * s + 64],
                                        in_=_r(ko3[row, B_IN3:B_IN3 + FH]))
                    nc.gpsimd.dma_start(out=L4[64 + s:65 + s, 32 * s:32 * s + 32],
                                        in_=_r(ko3[row, B_SH3:B_SH3 + FOUT]))
                    nc.gpsimd.dma_start(out=L4[66:67, 32 * s:32 * s + 32],
                                        in_=_r(ko3[row, B_OUT3:B_OUT3 + FOUT]))
                nc.gpsimd.dma_start(out=bmid[:, 0:1],
                                    in_=ko3[sa:sa + 2, B_MID3:B_MID3 + FH])
                pair_wts.append((L1, L2, L3, L4, bmid))

            for p in range(2):
                L1, L2, L3, L4, bmid = pair_wts[p]
                for jj in range(HW // 1024):
                    c0 = 1024 * jj
                    x_t = xinp.tile([XC, 1024], F32R, tag="x", name=f"x_{p}_{jj}")
                    nc.sync.dma_start(out=x_t, in_=_r(xs[p, :, c0:c0 + 1024]))
                    o_sb = outp.tile([FH, 1024], F32, tag="o", name=f"o_{p}_{jj}")
                    for h in range(2):
                        s0 = 512 * h
                        xv = x_t[:, s0:s0 + 512]
                        p1 = ps1p.tile([128, 512], F32, tag="p1",
                                       name=f"p1_{p}_{jj}_{h}")
                        nc.tensor.matmul(p1, lhsT=L1, rhs=xv, start=True, stop=True)
                        warm(f"wb_{p}_{jj}_{h}")
                        y1 = ysp.tile([128, 512], F32R, tag="y1",
                                      name=f"y1_{p}_{jj}_{h}")
                        nc.scalar.activation(out=y1, in_=p1, func=AF.Lrelu,
                                             bias=0.0, scale=1.0, alpha=0.01)
                        p2 = ps2p.tile([128, 512], F32, tag="p2",
                                       name=f"p2_{p}_{jj}_{h}")
                        nc.tensor.matmul(p2, lhsT=L2, rhs=y1, start=True, stop=True)
                        y2 = ysp.tile([128, 512], F32R, tag="y2",
                                      name=f"y2_{p}_{jj}_{h}")
                        nc.scalar.activation(out=y2, in_=p2, func=AF.Lrelu,
                                             bias=bmid[:, 0:1], scale=1.0,
                                             alpha=0.01)
                        p3 = ps3p.tile([FH, 512], F32, tag="p3",
                                       name=f"p3_{p}_{jj}_{h}")
                        nc.tensor.matmul(p3, lhsT=L3, rhs=y2, start=True, stop=False)
                        nc.tensor.matmul(p3, lhsT=L4, rhs=xv, start=False, stop=True)
                        nc.vector.tensor_copy(o_sb[:, s0:s0 + 512], p3)
                    nc.gpsimd.dma_start(out=out[p, :, c0:c0 + 1024], in_=o_sb)

    nc.compile()
    return nc


def _seg_perm(rows, cols):
    # new position (c, r) holds old flat index r*cols + c
    return np.arange(rows * cols).reshape(rows, cols).T.ravel()


def _perm():
    # permutation of ks columns so each conv kernel arrives transposed
    return np.concatenate([
        OFF_IN + _seg_perm(FH, FIN),
        OFF_MID + _seg_perm(FH, FH),
        OFF_OUT + _seg_perm(FOUT, FH),
        OFF_SHC + _seg_perm(FOUT, FIN),
        np.arange(OFF_B, K_TOT),
    ])


def _prep_in_maps(x, lat, W1, b1, W2, b2):
    x = np.ascontiguousarray(x, np.float32)
    lat = np.ascontiguousarray(lat, np.float32)
    W1 = np.ascontiguousarray(W1, np.float32)
    b1 = np.asarray(b1, np.float32)
    W2 = np.asarray(W2, np.float32)
    b2 = np.asarray(b2, np.float32)

    perm = _perm()
    W2p = W2[:, perm]
    b2p = b2[perm]
    latT = np.ascontiguousarray(lat.T)
    xr = x.reshape(B, FIN, HW)

    in_maps = []
    for i in range(N_CORES):
        sh = slice(i * SHARD, (i + 1) * SHARD)
        w1p = np.zeros((LAT, KPAD), np.float32)
        w1p[:, :SHARD] = W1[:, sh]
        b1p = np.zeros((KPAD,), np.float32)
        b1p[:SHARD] = b1[sh]
        b1p[SHARD] = 1.0  # the "ones" h-slot that carries b2
        w2a = np.zeros((KP, K_TOT), np.float32)
        w2a[:SHARD] = W2p[sh]
        if i == 0:
            w2a[SHARD] = b2p
        xsi = np.ones((2, XC, HW), np.float32)
        for p in range(2):
            xsi[p, 0:FIN] = xr[4 * i + 2 * p]
            xsi[p, FIN:2 * FIN] = xr[4 * i + 2 * p + 1]
        in_maps.append({
            "xs": xsi,
            "latT": latT,
            "w1s": w1p,
            "b1s": np.ascontiguousarray(b1p.reshape(NCH, 128).T),
            "w2s": w2a,
        })
    return in_maps


def _run(in_maps, **kwargs):
    if "nc" not in _CACHE:
        _CACHE["nc"] = _build()
    return run_bass_kernel_spmd(_CACHE["nc"], in_maps,
                                core_ids=list(range(N_CORES)), **kwargs)


def _assemble(results):
    parts = [r["out"].reshape(BPC, FOUT, H, W) for r in results]
    return np.ascontiguousarray(np.concatenate(parts, axis=0))


def kernel(x, lat, W1, b1, W2, b2):
    in_maps = _prep_in_maps(x, lat, W1, b1, W2, b2)
    res = _run(in_maps)
    return _assemble(res.results)


# revision 8
# speedup vs baseline: 1.3081x; 1.0331x over previous
"""Trainium2 Bass kernel for nn_DynaResidualBlock (hypernetwork residual block).

Reference computation (B=32, LAT=256, FIN=FOUT=32, FH=64, H=W=128):
    h  = lat @ W1 + b1                       # [B, 9408]
    ks = h @ W2 + b2                         # [B, 9408]  (W2 is 9408x9408 = 354 MB)
    per-sample 1x1 convs with kernels/biases sliced out of ks:
    x_s = k_short(x) ; y = k_out(lrelu(k_mid(lrelu(k_in(x))))) + x_s

Sharding over 8 cores:
  - hypernet contraction dim (9408) split 1176-per-core: core i holds
    W1[:, shard_i] and W2[shard_i, :] and computes a partial ks for ALL
    32 samples; per-segment ReduceScatters (overlapped with the W2
    streaming) hand core i the summed ks rows for its own 4 samples.
  - conv phase is data-parallel: core i processes samples 4i..4i+3, packed
    as 2 sample-pairs with block-diagonal weight matrices so each 1x1 conv
    over a 512-pixel tile is a single PE matmul.

Implementation notes:
  - All matmuls run as float32r (FP22 multiply, fp32 accumulate): single
    PE pass at full rate vs fp32's two half-rate passes.
  - fp32r matmuls do not register as PE activity for the HAM clock gate,
    which would leave the PE throttled at 1.2 GHz; tiny bf16 "warmer"
    matmuls are interleaved to keep the 2.4 GHz clock.
  - W2's columns are permuted host-side so each generated conv kernel
    lands in SBUF already transposed into the PE's lhsT layout.
  - x gets 3 constant "ones" channels per pair so conv biases ride in as
    extra matmul rows (b_in, b_short, b_out fold into the matmuls).
  - lat is passed pre-transposed; b2 rides as an extra W2 row on core 0.
"""

import contextlib

import numpy as np

import concourse.bacc as bacc
import concourse.mybir as mybir
import concourse.tile as tile
from concourse.bass_utils import run_bass_kernel_spmd

N_CORES = 8
B, LAT, FIN, FOUT, FH, H, W = 32, 256, 32, 32, 64, 128, 128
HW = H * W
K_IN, K_MID, K_OUT, K_SH = FH * FIN, FH * FH, FOUT * FH, FOUT * FIN
K_TOT = K_IN + K_MID + K_OUT + K_SH + FH + FH + FOUT + FOUT  # 9408
SHARD = K_TOT // N_CORES  # 1176 hypernet columns per core
KP = SHARD + 1            # + one bias row (b2, on core 0 only)
KPAD = 1280               # h length padded to 10 chunks of 128
NCH = KPAD // 128         # 10
BPC = B // N_CORES        # 4 samples per core
XC = 2 * FIN + 3          # 67 = 2x32 x-channels + 3 ones-channels per pair
F32 = mybir.dt.float32
F32R = mybir.dt.float32r  # FP22 multiplies at full PE rate, fp32 accumulate
BF16 = mybir.dt.bfloat16


def _r(ap):
    return ap.bitcast(F32R)


OFF_IN, OFF_MID = 0, K_IN
OFF_OUT, OFF_SHC = K_IN + K_MID, K_IN + K_MID + K_OUT
OFF_B = OFF_SHC + K_SH  # 9216: b_in 64 | b_mid 64 | b_out 32 | b_short 32

# phase-A ks column groups (512-multiples; last is the 1216 tail)
GROUPS = [(0, 2048), (2048, 2048), (4096, 2048), (6144, 2048), (8192, 1216)]
# two AllToAll exchanges: half a = k_in + k_mid (cols 0:6144, after group 2),
# half b = k_out + k_short + biases (cols 6144:9408, after group 4).
A2A_SPEC = [(0, 6144), (6144, 3264)]
A2A_AFTER_GROUP = {2: 0, 4: 1}
# offsets inside half b
B_OUT0 = 2048            # k_out is [0:2048] of half b... (k_out starts at 6144)
L4_OFF = 2048            # k_short at [2048:3072] of half b
B_IN3, B_MID3, B_OUT3, B_SH3 = 3072, 3136, 3200, 3232
FP16 = mybir.dt.float16

_CACHE: dict = {}


def _build():
    nc = bacc.Bacc("TRN2", target_bir_lowering=False, num_devices=N_CORES)
    AF = mybir.ActivationFunctionType

    xs = nc.dram_tensor("xs", [2, XC, HW], F32, kind="ExternalInput")
    latT = nc.dram_tensor("latT", [LAT, B], F32, kind="ExternalInput")
    w1s = nc.dram_tensor("w1s", [LAT, KPAD], F32, kind="ExternalInput")
    b1s = nc.dram_tensor("b1s", [128, NCH], F32, kind="ExternalInput")
    w2s = nc.dram_tensor("w2s", [KP, K_TOT], F32, kind="ExternalInput")
    out = nc.dram_tensor("out", [2, 2 * FOUT, HW], F32, kind="ExternalOutput")

    zeros_dram = nc.inline_tensor(np.zeros((128, 128), np.float32), name="zconst")
    a2a_in = [nc.dram_tensor(f"a2a_in{g}", [B, w], F32)
              for g, (_, w) in enumerate(A2A_SPEC)]
    a2a_out = [nc.dram_tensor(f"a2a_out{g}", [B, w], F32)
               for g, (_, w) in enumerate(A2A_SPEC)]
    ks_own = [nc.dram_tensor(f"ks_own{g}", [BPC, w], F32)
              for g, (_, w) in enumerate(A2A_SPEC)]

    with tile.TileContext(nc) as tc, contextlib.ExitStack() as ctx:
        sing = ctx.enter_context(tc.tile_pool(name="sing", bufs=1))
        a2ap = ctx.enter_context(tc.tile_pool(name="a2ap", bufs=8))

        def a2a_reduce(idx):
            """Exchange partials (AllToAll) and tree-sum the 8 received
            chunks on DVE; result lands in ks_own[idx]."""
            _, w = A2A_SPEC[idx]
            wf = BPC * w // 128
            nc.gpsimd.collective_compute(
                "AllToAll", mybir.AluOpType.bypass,
                replica_groups=[list(range(N_CORES))],
                ins=[a2a_in[idx][:, :].opt()], outs=[a2a_out[idx][:, :].opt()])
            ch = []
            for c in range(N_CORES):
                t = a2ap.tile([128, 192], F32, tag="a2c", name=f"a2c_{idx}_{c}")
                nc.gpsimd.dma_start(out=t[:, 0:wf],
                                    in_=a2a_out[idx][BPC * c:BPC * (c + 1), :])
                ch.append(t)
            for i, j in [(0, 1), (2, 3), (4, 5), (6, 7), (0, 2), (4, 6), (0, 4)]:
                nc.vector.tensor_add(ch[i][:, 0:wf], ch[i][:, 0:wf], ch[j][:, 0:wf])
            nc.sync.dma_start(out=ks_own[idx][:, :], in_=ch[0][:, 0:wf])

        # ---------------- Phase A: hypernet (partial ks for all B) --------
        with tc.tile_pool(name="psA", bufs=2, space="PSUM") as psA, \
             tc.tile_pool(name="psK", bufs=6, space="PSUM") as psK, \
             tc.tile_pool(name="w2pool", bufs=6) as w2pool, \
             tc.tile_pool(name="kspool", bufs=2) as kspool:
            latT_sb = sing.tile([128, 2 * B], F32R)
            w1_sb = sing.tile([128, 2 * KPAD], F32R)
            b1_sb = sing.tile([128, NCH], F32)
            hT_sb = sing.tile([128, NCH * B], F32R)
            for l in range(2):
                nc.sync.dma_start(out=latT_sb[:, l * B:(l + 1) * B],
                                  in_=_r(latT[128 * l:128 * (l + 1), :]))
                nc.sync.dma_start(out=w1_sb[:, l * KPAD:(l + 1) * KPAD],
                                  in_=_r(w1s[128 * l:128 * (l + 1), :]))
            nc.sync.dma_start(out=b1_sb[:, :], in_=b1s[:, :])

            # hT[kcol, b] = sum_l W1[l, kcol] * lat[b, l]  (+ b1[kcol])
            for c in range(NCH):
                ph = psA.tile([128, B], F32, tag="ph", name=f"ph{c}")
                for l in range(2):
                    nc.tensor.matmul(
                        ph,
                        lhsT=w1_sb[:, l * KPAD + 128 * c: l * KPAD + 128 * c + 128],
                        rhs=latT_sb[:, l * B:(l + 1) * B],
                        start=(l == 0), stop=(l == 1))
                nc.scalar.activation(out=hT_sb[:, c * B:(c + 1) * B], in_=ph,
                                     func=AF.Identity, bias=b1_sb[:, c:c + 1],
                                     scale=1.0)

            # partial ks[b, n] = sum_k h[b, k] * W2s[k, n] ; W2s streamed
            for g, (g0, gw) in enumerate(GROUPS):
                nts = [(t, min(512, gw - t)) for t in range(0, gw, 512)]
                pts = [psK.tile([B, 512], F32, tag="pk", name=f"pk_{g0}_{t0}")
                       for t0, _ in nts]
                for c in range(NCH):
                    kc = 128 if c < NCH - 1 else KP - 128 * (NCH - 1)
                    w2t = w2pool.tile([128, 2048], F32R, tag="w2t",
                                      name=f"w2t_{g0}_{c}")
                    dma_eng = nc.sync if c % 2 == 0 else nc.scalar
                    dma_eng.dma_start(out=w2t[0:kc, 0:gw],
                                      in_=_r(w2s[128 * c:128 * c + kc, g0:g0 + gw]))
                    for ti, (t0, tw) in enumerate(nts):
                        nc.tensor.matmul(
                            pts[ti][:, 0:tw],
                            lhsT=hT_sb[0:kc, c * B:(c + 1) * B],
                            rhs=w2t[0:kc, t0:t0 + tw],
                            start=(c == 0), stop=(c == NCH - 1))
                ks_sb = kspool.tile([B, 2048], F32, tag="ks_sb", name=f"ks_{g0}")
                for ti, (t0, tw) in enumerate(nts):
                    nc.scalar.activation(out=ks_sb[:, t0:t0 + tw],
                                         in_=pts[ti][:, 0:tw], func=AF.Copy)
                # store into the right A2A input tensor
                a_i = 0 if g0 < 6144 else 1
                r0, _ = A2A_SPEC[a_i]
                nc.sync.dma_start(out=a2a_in[a_i][:, g0 - r0:g0 - r0 + gw],
                                  in_=ks_sb[:, 0:gw])
                if g in A2A_AFTER_GROUP:
                    a2a_reduce(A2A_AFTER_GROUP[g])

        koa, kob = ks_own

        # ---------------- Phase B: per-sample 1x1 convs (fp16) ------------
        with tc.tile_pool(name="wts", bufs=1) as wts, \
             tc.tile_pool(name="ps1", bufs=2, space="PSUM") as ps1p, \
             tc.tile_pool(name="ps2", bufs=1, space="PSUM") as ps2p, \
             tc.tile_pool(name="ps3", bufs=1, space="PSUM") as ps3p, \
             tc.tile_pool(name="xin", bufs=3) as xinp, \
             tc.tile_pool(name="ys", bufs=3) as ysp, \
             tc.tile_pool(name="outp", bufs=3) as outp:
            pair_wts = []
            for p in range(2):
                sa = 2 * p
                L1 = wts.tile([XC, 128], FP16, name=f"L1_{p}")
                L2 = wts.tile([128, 128], FP16, name=f"L2_{p}")
                L3 = wts.tile([128, FH], FP16, name=f"L3_{p}")
                L4 = wts.tile([XC, FH], FP16, name=f"L4_{p}")
                bmid = wts.tile([128, 1], F32, name=f"bm_{p}")
                for t in (L1, L2, L3, L4):
                    pp, ff = t.shape
                    nc.gpsimd.dma_start(out=t, in_=zeros_dram[0:pp, 0:ff])
                for s in range(2):
                    row = sa + s
                    # conv kernels, already transposed to lhsT layout by the
                    # host-side W2 column permutation (fp32 -> fp16 cast DMA)
                    nc.gpsimd.dma_start(
                        out=L1[32 * s:32 * s + 32, 64 * s:64 * s + 64],
                        in_=koa[row, 0:K_IN].rearrange("(i o) -> i o", i=FIN))
                    nc.gpsimd.dma_start(
                        out=L2[64 * s:64 * s + 64, 64 * s:64 * s + 64],
                        in_=koa[row, K_IN:K_IN + K_MID]
                        .rearrange("(i o) -> i o", i=FH))
                    nc.gpsimd.dma_start(
                        out=L3[64 * s:64 * s + 64, 32 * s:32 * s + 32],
                        in_=kob[row, 0:K_OUT].rearrange("(i o) -> i o", i=FH))
                    nc.gpsimd.dma_start(
                        out=L4[32 * s:32 * s + 32, 32 * s:32 * s + 32],
                        in_=kob[row, L4_OFF:L4_OFF + K_SH]
                        .rearrange("(i o) -> i o", i=FIN))
                    # bias rows, consumed via the ones-channels of xs
                    nc.gpsimd.dma_start(out=L1[64 + s:65 + s, 64 * s:64 * s + 64],
                                        in_=kob[row, B_IN3:B_IN3 + FH])
                    nc.gpsimd.dma_start(out=L4[64 + s:65 + s, 32 * s:32 * s + 32],
                                        in_=kob[row, B_SH3:B_SH3 + FOUT])
                    nc.gpsimd.dma_start(out=L4[66:67, 32 * s:32 * s + 32],
                                        in_=kob[row, B_OUT3:B_OUT3 + FOUT])
                nc.gpsimd.dma_start(out=bmid[:, 0:1],
                                    in_=kob[sa:sa + 2, B_MID3:B_MID3 + FH])
                pair_wts.append((L1, L2, L3, L4, bmid))

            (L1a, L2a, L3a, L4a, bma), (L1b, L2b, L3b, L4b, bmb) = pair_wts
            for jj in range(HW // 1024):
                c0 = 1024 * jj
                x0 = xinp.tile([XC, 1024], FP16, tag="x0", name=f"x0_{jj}")
                x1 = xinp.tile([XC, 1024], FP16, tag="x1", name=f"x1_{jj}")
                nc.gpsimd.dma_start(out=x0, in_=xs[0, :, c0:c0 + 1024])
                nc.gpsimd.dma_start(out=x1, in_=xs[1, :, c0:c0 + 1024])
                o0 = outp.tile([FH, 1024], F32, tag="o0", name=f"o0_{jj}")
                o1 = outp.tile([FH, 1024], F32, tag="o1", name=f"o1_{jj}")
                for h in range(2):
                    s0 = 512 * h
                    xv0, xv1 = x0[:, s0:s0 + 512], x1[:, s0:s0 + 512]
                    p1 = ps1p.tile([128, 1024], F32, tag="p1",
                                   name=f"p1_{jj}_{h}")
                    nc.tensor.matmul(p1[:, 0:512], lhsT=L1a, rhs=xv0,
                                     start=True, stop=True)
                    nc.tensor.matmul(p1[:, 512:1024], lhsT=L1b, rhs=xv1,
                                     start=True, stop=True)
                    y1 = ysp.tile([128, 1024], FP16, tag="y1",
                                  name=f"y1_{jj}_{h}")
                    nc.scalar.activation(out=y1, in_=p1, func=AF.Lrelu,
                                         bias=0.0, scale=1.0, alpha=0.01)
                    p2 = ps2p.tile([128, 1024], F32, tag="p2",
                                   name=f"p2_{jj}_{h}")
                    nc.tensor.matmul(p2[:, 0:512], lhsT=L2a, rhs=y1[:, 0:512],
                                     start=True, stop=True)
                    nc.tensor.matmul(p2[:, 512:1024], lhsT=L2b,
                                     rhs=y1[:, 512:1024], start=True, stop=True)
                    y2 = ysp.tile([128, 1024], FP16, tag="y2",
                                  name=f"y2_{jj}_{h}")
                    nc.scalar.activation(out=y2[:, 0:512], in_=p2[:, 0:512],
                                         func=AF.Lrelu, bias=bma[:, 0:1],
                                         scale=1.0, alpha=0.01)
                    nc.scalar.activation(out=y2[:, 512:1024], in_=p2[:, 512:1024],
                                         func=AF.Lrelu, bias=bmb[:, 0:1],
                                         scale=1.0, alpha=0.01)
                    p3 = ps3p.tile([FH, 1024], F32, tag="p3",
                                   name=f"p3_{jj}_{h}")
                    nc.tensor.matmul(p3[:, 0:512], lhsT=L3a, rhs=y2[:, 0:512],
                                     start=True, stop=False)
                    nc.tensor.matmul(p3[:, 0:512], lhsT=L4a, rhs=xv0,
                                     start=False, stop=True)
                    nc.tensor.matmul(p3[:, 512:1024], lhsT=L3b,
                                     rhs=y2[:, 512:1024], start=True, stop=False)
                    nc.tensor.matmul(p3[:, 512:1024], lhsT=L4b, rhs=xv1,
                                     start=False, stop=True)
                    nc.vector.tensor_copy(o0[:, s0:s0 + 512], p3[:, 0:512])
                    nc.vector.tensor_copy(o1[:, s0:s0 + 512], p3[:, 512:1024])
                nc.sync.dma_start(out=out[0, :, c0:c0 + 1024], in_=o0)
                nc.sync.dma_start(out=out[1, :, c0:c0 + 1024], in_=o1)

    nc.compile()
    return nc


def _seg_perm(rows, cols):
    # new position (c, r) holds old flat index r*cols + c
    return np.arange(rows * cols).reshape(rows, cols).T.ravel()


def _perm():
    # permutation of ks columns so each conv kernel arrives transposed
    return np.concatenate([
        OFF_IN + _seg_perm(FH, FIN),
        OFF_MID + _seg_perm(FH, FH),
        OFF_OUT + _seg_perm(FOUT, FH),
        OFF_SHC + _seg_perm(FOUT, FIN),
        np.arange(OFF_B, K_TOT),
    ])


def _prep_in_maps(x, lat, W1, b1, W2, b2):
    x = np.ascontiguousarray(x, np.float32)
    lat = np.ascontiguousarray(lat, np.float32)
    W1 = np.ascontiguousarray(W1, np.float32)
    b1 = np.asarray(b1, np.float32)
    W2 = np.asarray(W2, np.float32)
    b2 = np.asarray(b2, np.float32)

    perm = _perm()
    W2p = W2[:, perm]
    b2p = b2[perm]
    latT = np.ascontiguousarray(lat.T)
    xr = x.reshape(B, FIN, HW)

    in_maps = []
    for i in range(N_CORES):
        sh = slice(i * SHARD, (i + 1) * SHARD)
        w1p = np.zeros((LAT, KPAD), np.float32)
        w1p[:, :SHARD] = W1[:, sh]
        b1p = np.zeros((KPAD,), np.float32)
        b1p[:SHARD] = b1[sh]
        b1p[SHARD] = 1.0  # the "ones" h-slot that carries b2
        w2a = np.zeros((KP, K_TOT), np.float32)
        w2a[:SHARD] = W2p[sh]
        if i == 0:
            w2a[SHARD] = b2p
        xsi = np.ones((2, XC, HW), np.float32)
        for p in range(2):
            xsi[p, 0:FIN] = xr[4 * i + 2 * p]
            xsi[p, FIN:2 * FIN] = xr[4 * i + 2 * p + 1]
        in_maps.append({
            "xs": xsi,
            "latT": latT,
            "w1s": w1p,
            "b1s": np.ascontiguousarray(b1p.reshape(NCH, 128).T),
            "w2s": w2a,
        })
    return in_maps


def _run(in_maps, **kwargs):
    if "nc" not in _CACHE:
        _CACHE["nc"] = _build()
    return run_bass_kernel_spmd(_CACHE["nc"], in_maps,
                                core_ids=list(range(N_CORES)), **kwargs)


def _assemble(results):
    parts = [r["out"].reshape(BPC, FOUT, H, W) for r in results]
    return np.ascontiguousarray(np.concatenate(parts, axis=0))


def kernel(x, lat, W1, b1, W2, b2):
    in_maps = _prep_in_maps(x, lat, W1, b1, W2, b2)
    res = _run(in_maps)
    return _assemble(res.results)


# revision 9
# speedup vs baseline: 1.4630x; 1.1184x over previous
"""Trainium2 Bass kernel for nn_DynaResidualBlock (hypernetwork residual block).

Reference computation (B=32, LAT=256, FIN=FOUT=32, FH=64, H=W=128):
    h  = lat @ W1 + b1                       # [B, 9408]
    ks = h @ W2 + b2                         # [B, 9408]  (W2 is 9408x9408 = 354 MB)
    per-sample 1x1 convs with kernels/biases sliced out of ks:
    x_s = k_short(x) ; y = k_out(lrelu(k_mid(lrelu(k_in(x))))) + x_s

Sharding over 8 cores:
  - hypernet contraction dim (9408) split 1176-per-core: core i holds
    W1[:, shard_i] and W2[shard_i, :] and computes a partial ks for ALL
    32 samples; per-segment ReduceScatters (overlapped with the W2
    streaming) hand core i the summed ks rows for its own 4 samples.
  - conv phase is data-parallel: core i processes samples 4i..4i+3, packed
    as 2 sample-pairs with block-diagonal weight matrices so each 1x1 conv
    over a 512-pixel tile is a single PE matmul.

Implementation notes:
  - All matmuls run as float32r (FP22 multiply, fp32 accumulate): single
    PE pass at full rate vs fp32's two half-rate passes.
  - fp32r matmuls do not register as PE activity for the HAM clock gate,
    which would leave the PE throttled at 1.2 GHz; tiny bf16 "warmer"
    matmuls are interleaved to keep the 2.4 GHz clock.
  - W2's columns are permuted host-side so each generated conv kernel
    lands in SBUF already transposed into the PE's lhsT layout.
  - x gets 3 constant "ones" channels per pair so conv biases ride in as
    extra matmul rows (b_in, b_short, b_out fold into the matmuls).
  - lat is passed pre-transposed; b2 rides as an extra W2 row on core 0.
"""

import contextlib

import numpy as np

import concourse.bacc as bacc
import concourse.mybir as mybir
import concourse.tile as tile
from concourse.bass_utils import run_bass_kernel_spmd

N_CORES = 8
B, LAT, FIN, FOUT, FH, H, W = 32, 256, 32, 32, 64, 128, 128
HW = H * W
K_IN, K_MID, K_OUT, K_SH = FH * FIN, FH * FH, FOUT * FH, FOUT * FIN
K_TOT = K_IN + K_MID + K_OUT + K_SH + FH + FH + FOUT + FOUT  # 9408
SHARD = K_TOT // N_CORES  # 1176 hypernet columns per core
KP = SHARD + 1            # + one bias row (b2, on core 0 only)
KPAD = 1280               # h length padded to 10 chunks of 128
NCH = KPAD // 128         # 10
BPC = B // N_CORES        # 4 samples per core
XC = 2 * FIN + 3          # 67 = 2x32 x-channels + 3 ones-channels per pair
F32 = mybir.dt.float32
F32R = mybir.dt.float32r  # FP22 multiplies at full PE rate, fp32 accumulate
BF16 = mybir.dt.bfloat16


def _r(ap):
    return ap.bitcast(F32R)


OFF_IN, OFF_MID = 0, K_IN
OFF_OUT, OFF_SHC = K_IN + K_MID, K_IN + K_MID + K_OUT
OFF_B = OFF_SHC + K_SH  # 9216: b_in 64 | b_mid 64 | b_out 32 | b_short 32

# phase-A ks column groups (512-multiples; last is the 1216 tail)
GROUPS = [(0, 2048), (2048, 2048), (4096, 2048), (6144, 2048), (8192, 1216)]
# two AllToAll exchanges: half a = k_in + k_mid (cols 0:6144, after group 2),
# half b = k_out + k_short + biases (cols 6144:9408, after group 4).
A2A_SPEC = [(0, 6144), (6144, 3264)]
A2A_AFTER_GROUP = {2: 0, 4: 1}
# offsets inside half b
B_OUT0 = 2048            # k_out is [0:2048] of half b... (k_out starts at 6144)
L4_OFF = 2048            # k_short at [2048:3072] of half b
B_IN3, B_MID3, B_OUT3, B_SH3 = 3072, 3136, 3200, 3232
FP16 = mybir.dt.float16

_CACHE: dict = {}


def _build():
    nc = bacc.Bacc("TRN2", target_bir_lowering=False, num_devices=N_CORES)
    AF = mybir.ActivationFunctionType

    xs = nc.dram_tensor("xs", [2, XC, HW], FP16, kind="ExternalInput")
    latT = nc.dram_tensor("latT", [LAT, B], F32, kind="ExternalInput")
    w1s = nc.dram_tensor("w1s", [LAT, KPAD], F32, kind="ExternalInput")
    b1s = nc.dram_tensor("b1s", [128, NCH], F32, kind="ExternalInput")
    w2s = nc.dram_tensor("w2s", [KP, K_TOT], F32, kind="ExternalInput")
    out = nc.dram_tensor("out", [2, 2 * FOUT, HW], F32, kind="ExternalOutput")

    zeros16 = nc.inline_tensor(np.zeros((128, 128), np.float16), name="zconst16")
    a2a_in = [nc.dram_tensor(f"a2a_in{g}", [B, w], F32)
              for g, (_, w) in enumerate(A2A_SPEC)]
    a2a_out = [nc.dram_tensor(f"a2a_out{g}", [B, w], F32)
               for g, (_, w) in enumerate(A2A_SPEC)]
    ks_own = [nc.dram_tensor(f"ks_own{g}", [BPC, w], F32)
              for g, (_, w) in enumerate(A2A_SPEC)]

    with tile.TileContext(nc) as tc, contextlib.ExitStack() as ctx:
        sing = ctx.enter_context(tc.tile_pool(name="sing", bufs=1))
        a2ap = ctx.enter_context(tc.tile_pool(name="a2ap", bufs=8))

        def a2a_reduce(idx):
            """Exchange partials (AllToAll) and tree-sum the 8 received
            chunks on DVE; result lands in ks_own[idx]."""
            _, w = A2A_SPEC[idx]
            wf = BPC * w // 128
            nc.gpsimd.collective_compute(
                "AllToAll", mybir.AluOpType.bypass,
                replica_groups=[list(range(N_CORES))],
                ins=[a2a_in[idx][:, :].opt()], outs=[a2a_out[idx][:, :].opt()])
            ch = []
            for c in range(N_CORES):
                t = a2ap.tile([128, 192], F32, tag="a2c", name=f"a2c_{idx}_{c}")
                nc.sync.dma_start(out=t[:, 0:wf],
                                  in_=a2a_out[idx][BPC * c:BPC * (c + 1), :])
                ch.append(t)
            for i, j in [(0, 1), (2, 3), (4, 5), (6, 7), (0, 2), (4, 6), (0, 4)]:
                nc.vector.tensor_add(ch[i][:, 0:wf], ch[i][:, 0:wf], ch[j][:, 0:wf])
            nc.sync.dma_start(out=ks_own[idx][:, :], in_=ch[0][:, 0:wf])

        # ---------------- Phase A: hypernet (partial ks for all B) --------
        with tc.tile_pool(name="psA", bufs=2, space="PSUM") as psA, \
             tc.tile_pool(name="psK", bufs=6, space="PSUM") as psK, \
             tc.tile_pool(name="w2pool", bufs=6) as w2pool, \
             tc.tile_pool(name="kspool", bufs=2) as kspool:
            latT_sb = sing.tile([128, 2 * B], F32R)
            w1_sb = sing.tile([128, 2 * KPAD], F32R)
            b1_sb = sing.tile([128, NCH], F32)
            hT_sb = sing.tile([128, NCH * B], F32R)
            for l in range(2):
                nc.sync.dma_start(out=latT_sb[:, l * B:(l + 1) * B],
                                  in_=_r(latT[128 * l:128 * (l + 1), :]))
                nc.sync.dma_start(out=w1_sb[:, l * KPAD:(l + 1) * KPAD],
                                  in_=_r(w1s[128 * l:128 * (l + 1), :]))
            nc.sync.dma_start(out=b1_sb[:, :], in_=b1s[:, :])

            # hT[kcol, b] = sum_l W1[l, kcol] * lat[b, l]  (+ b1[kcol])
            for c in range(NCH):
                ph = psA.tile([128, B], F32, tag="ph", name=f"ph{c}")
                for l in range(2):
                    nc.tensor.matmul(
                        ph,
                        lhsT=w1_sb[:, l * KPAD + 128 * c: l * KPAD + 128 * c + 128],
                        rhs=latT_sb[:, l * B:(l + 1) * B],
                        start=(l == 0), stop=(l == 1))
                nc.scalar.activation(out=hT_sb[:, c * B:(c + 1) * B], in_=ph,
                                     func=AF.Identity, bias=b1_sb[:, c:c + 1],
                                     scale=1.0)

            # partial ks[b, n] = sum_k h[b, k] * W2s[k, n] ; W2s streamed
            for g, (g0, gw) in enumerate(GROUPS):
                nts = [(t, min(512, gw - t)) for t in range(0, gw, 512)]
                pts = [psK.tile([B, 512], F32, tag="pk", name=f"pk_{g0}_{t0}")
                       for t0, _ in nts]
                for c in range(NCH):
                    kc = 128 if c < NCH - 1 else KP - 128 * (NCH - 1)
                    w2t = w2pool.tile([128, 2048], F32R, tag="w2t",
                                      name=f"w2t_{g0}_{c}")
                    dma_eng = nc.sync if c % 2 == 0 else nc.scalar
                    dma_eng.dma_start(out=w2t[0:kc, 0:gw],
                                      in_=_r(w2s[128 * c:128 * c + kc, g0:g0 + gw]))
                    for ti, (t0, tw) in enumerate(nts):
                        nc.tensor.matmul(
                            pts[ti][:, 0:tw],
                            lhsT=hT_sb[0:kc, c * B:(c + 1) * B],
                            rhs=w2t[0:kc, t0:t0 + tw],
                            start=(c == 0), stop=(c == NCH - 1))
                ks_sb = kspool.tile([B, 2048], F32, tag="ks_sb", name=f"ks_{g0}")
                for ti, (t0, tw) in enumerate(nts):
                    nc.scalar.activation(out=ks_sb[:, t0:t0 + tw],
                                         in_=pts[ti][:, 0:tw], func=AF.Copy)
                # store into the right A2A input tensor
                a_i = 0 if g0 < 6144 else 1
                r0, _ = A2A_SPEC[a_i]
                nc.sync.dma_start(out=a2a_in[a_i][:, g0 - r0:g0 - r0 + gw],
                                  in_=ks_sb[:, 0:gw])
                if g in A2A_AFTER_GROUP:
                    a2a_reduce(A2A_AFTER_GROUP[g])

        koa, kob = ks_own

        # ---------------- Phase B: per-sample 1x1 convs (fp16) ------------
        with tc.tile_pool(name="wts", bufs=1) as wts, \
             tc.tile_pool(name="ps1", bufs=2, space="PSUM") as ps1p, \
             tc.tile_pool(name="ps2", bufs=1, space="PSUM") as ps2p, \
             tc.tile_pool(name="ps3", bufs=1, space="PSUM") as ps3p, \
             tc.tile_pool(name="xin", bufs=3) as xinp, \
             tc.tile_pool(name="ys", bufs=3) as ysp, \
             tc.tile_pool(name="outp", bufs=3) as outp:
            pair_wts = []
            for p in range(2):
                sa = 2 * p
                L1 = wts.tile([XC, 128], FP16, name=f"L1_{p}")
                L2 = wts.tile([128, 128], FP16, name=f"L2_{p}")
                L3 = wts.tile([128, FH], FP16, name=f"L3_{p}")
                L4 = wts.tile([XC, FH], FP16, name=f"L4_{p}")
                bmid = wts.tile([128, 1], F32, name=f"bm_{p}")
                for t in (L1, L2, L3, L4):
                    pp, ff = t.shape
                    nc.gpsimd.dma_start(out=t, in_=zeros16[0:pp, 0:ff])
                for s in range(2):
                    row = sa + s
                    # conv kernels, already transposed to lhsT layout by the
                    # host-side W2 column permutation (fp32 -> fp16 cast DMA)
                    nc.gpsimd.dma_start(
                        out=L1[32 * s:32 * s + 32, 64 * s:64 * s + 64],
                        in_=koa[row, 0:K_IN].rearrange("(i o) -> i o", i=FIN))
                    nc.gpsimd.dma_start(
                        out=L2[64 * s:64 * s + 64, 64 * s:64 * s + 64],
                        in_=koa[row, K_IN:K_IN + K_MID]
                        .rearrange("(i o) -> i o", i=FH))
                    nc.gpsimd.dma_start(
                        out=L3[64 * s:64 * s + 64, 32 * s:32 * s + 32],
                        in_=kob[row, 0:K_OUT].rearrange("(i o) -> i o", i=FH))
                    nc.gpsimd.dma_start(
                        out=L4[32 * s:32 * s + 32, 32 * s:32 * s + 32],
                        in_=kob[row, L4_OFF:L4_OFF + K_SH]
                        .rearrange("(i o) -> i o", i=FIN))
                    # bias rows, consumed via the ones-channels of xs
                    nc.gpsimd.dma_start(out=L1[64 + s:65 + s, 64 * s:64 * s + 64],
                                        in_=kob[row, B_IN3:B_IN3 + FH])
                    nc.gpsimd.dma_start(out=L4[64 + s:65 + s, 32 * s:32 * s + 32],
                                        in_=kob[row, B_SH3:B_SH3 + FOUT])
                    nc.gpsimd.dma_start(out=L4[66:67, 32 * s:32 * s + 32],
                                        in_=kob[row, B_OUT3:B_OUT3 + FOUT])
                nc.gpsimd.dma_start(out=bmid[:, 0:1],
                                    in_=kob[sa:sa + 2, B_MID3:B_MID3 + FH])
                pair_wts.append((L1, L2, L3, L4, bmid))

            (L1a, L2a, L3a, L4a, bma), (L1b, L2b, L3b, L4b, bmb) = pair_wts
            for jj in range(HW // 1024):
                c0 = 1024 * jj
                x0 = xinp.tile([XC, 1024], FP16, tag="x0", name=f"x0_{jj}")
                x1 = xinp.tile([XC, 1024], FP16, tag="x1", name=f"x1_{jj}")
                nc.sync.dma_start(out=x0, in_=xs[0, :, c0:c0 + 1024])
                nc.sync.dma_start(out=x1, in_=xs[1, :, c0:c0 + 1024])
                o0 = outp.tile([FH, 1024], F32, tag="o0", name=f"o0_{jj}")
                o1 = outp.tile([FH, 1024], F32, tag="o1", name=f"o1_{jj}")
                for h in range(2):
                    s0 = 512 * h
                    xv0, xv1 = x0[:, s0:s0 + 512], x1[:, s0:s0 + 512]
                    p1 = ps1p.tile([128, 1024], F32, tag="p1",
                                   name=f"p1_{jj}_{h}")
                    nc.tensor.matmul(p1[:, 0:512], lhsT=L1a, rhs=xv0,
                                     start=True, stop=True)
                    nc.tensor.matmul(p1[:, 512:1024], lhsT=L1b, rhs=xv1,
                                     start=True, stop=True)
                    y1 = ysp.tile([128, 1024], FP16, tag="y1",
                                  name=f"y1_{jj}_{h}")
                    nc.scalar.activation(out=y1, in_=p1, func=AF.Lrelu,
                                         bias=0.0, scale=1.0, alpha=0.01)
                    p2 = ps2p.tile([128, 1024], F32, tag="p2",
                                   name=f"p2_{jj}_{h}")
                    nc.tensor.matmul(p2[:, 0:512], lhsT=L2a, rhs=y1[:, 0:512],
                                     start=True, stop=True)
                    nc.tensor.matmul(p2[:, 512:1024], lhsT=L2b,
                                     rhs=y1[:, 512:1024], start=True, stop=True)
                    y2 = ysp.tile([128, 1024], FP16, tag="y2",
                                  name=f"y2_{jj}_{h}")
                    nc.scalar.activation(out=y2[:, 0:512], in_=p2[:, 0:512],
                                         func=AF.Lrelu, bias=bma[:, 0:1],
                                         scale=1.0, alpha=0.01)
                    nc.scalar.activation(out=y2[:, 512:1024], in_=p2[:, 512:1024],
                                         func=AF.Lrelu, bias=bmb[:, 0:1],
                                         scale=1.0, alpha=0.01)
                    p3 = ps3p.tile([FH, 1024], F32, tag="p3",
                                   name=f"p3_{jj}_{h}")
                    nc.tensor.matmul(p3[:, 0:512], lhsT=L3a, rhs=y2[:, 0:512],
                                     start=True, stop=False)
                    nc.tensor.matmul(p3[:, 0:512], lhsT=L4a, rhs=xv0,
                                     start=False, stop=True)
                    nc.tensor.matmul(p3[:, 512:1024], lhsT=L3b,
                                     rhs=y2[:, 512:1024], start=True, stop=False)
                    nc.tensor.matmul(p3[:, 512:1024], lhsT=L4b, rhs=xv1,
                                     start=False, stop=True)
                    nc.vector.tensor_copy(o0[:, s0:s0 + 512], p3[:, 0:512])
                    nc.vector.tensor_copy(o1[:, s0:s0 + 512], p3[:, 512:1024])
                nc.sync.dma_start(out=out[0, :, c0:c0 + 1024], in_=o0)
                nc.sync.dma_start(out=out[1, :, c0:c0 + 1024], in_=o1)

    nc.compile()
    return nc


def _seg_perm(rows, cols):
    # new position (c, r) holds old flat index r*cols + c
    return np.arange(rows * cols).reshape(rows, cols).T.ravel()


def _perm():
    # permutation of ks columns so each conv kernel arrives transposed
    return np.concatenate([
        OFF_IN + _seg_perm(FH, FIN),
        OFF_MID + _seg_perm(FH, FH),
        OFF_OUT + _seg_perm(FOUT, FH),
        OFF_SHC + _seg_perm(FOUT, FIN),
        np.arange(OFF_B, K_TOT),
    ])


def _prep_in_maps(x, lat, W1, b1, W2, b2):
    x = np.ascontiguousarray(x, np.float32)
    lat = np.ascontiguousarray(lat, np.float32)
    W1 = np.ascontiguousarray(W1, np.float32)
    b1 = np.asarray(b1, np.float32)
    W2 = np.asarray(W2, np.float32)
    b2 = np.asarray(b2, np.float32)

    perm = _perm()
    W2p = W2[:, perm]
    b2p = b2[perm]
    latT = np.ascontiguousarray(lat.T)
    xr = x.reshape(B, FIN, HW)

    in_maps = []
    for i in range(N_CORES):
        sh = slice(i * SHARD, (i + 1) * SHARD)
        w1p = np.zeros((LAT, KPAD), np.float32)
        w1p[:, :SHARD] = W1[:, sh]
        b1p = np.zeros((KPAD,), np.float32)
        b1p[:SHARD] = b1[sh]
        b1p[SHARD] = 1.0  # the "ones" h-slot that carries b2
        w2a = np.zeros((KP, K_TOT), np.float32)
        w2a[:SHARD] = W2p[sh]
        if i == 0:
            w2a[SHARD] = b2p
        xsi = np.ones((2, XC, HW), np.float16)
        for p in range(2):
            xsi[p, 0:FIN] = xr[4 * i + 2 * p]
            xsi[p, FIN:2 * FIN] = xr[4 * i + 2 * p + 1]
        in_maps.append({
            "xs": xsi,
            "latT": latT,
            "w1s": w1p,
            "b1s": np.ascontiguousarray(b1p.reshape(NCH, 128).T),
            "w2s": w2a,
        })
    return in_maps


def _run(in_maps, **kwargs):
    if "nc" not in _CACHE:
        _CACHE["nc"] = _build()
    return run_bass_kernel_spmd(_CACHE["nc"], in_maps,
                                core_ids=list(range(N_CORES)), **kwargs)


def _assemble(results):
    parts = [r["out"].reshape(BPC, FOUT, H, W) for r in results]
    return np.ascontiguousarray(np.concatenate(parts, axis=0))


def kernel(x, lat, W1, b1, W2, b2):
    in_maps = _prep_in_maps(x, lat, W1, b1, W2, b2)
    res = _run(in_maps)
    return _assemble(res.results)
